# revision 1
# baseline (speedup 1.0000x reference)
"""Trainium2 Bass kernel for nn_CrossAttentionSequencePool.

Computation (see problem reference):
    x_before/x_after = exclusive prefix/suffix cummax of key rows (0 at boundary)
    x_key   = relu([key|x_before|x_after] @ k1_w.T + k1_b) @ k2_w.T + k2_b
    x_query = relu(query @ q1_w.T + q1_b) @ q2_w.T + q2_b
    res     = (x_query @ x_key.T) / 16                      # [1024, 32768] f32

Distribution: key rows sharded across 8 cores (4096 each), score matrix
sharded along n. Cross-shard cummax handled with per-shard seed vectors
(two-pass: shard maxima + exclusive scan over shards happen at input-prep
time; the local 4096-long scans run on-device via a custom DVE scan op).
Compute in fp16 with f32 PSUM accumulation; all tensors kept transposed
(features on partitions, sequence on the free dim).
"""

import json

import numpy as np

import concourse.bass as bass
import concourse.mybir as mybir
import concourse.tile as tile

# ---------------------------------------------------------------------------
# Patch 1: this container's walrus build accepts at most ONE semaphore wait
# per instruction; Tile freely emits several. Split extra waits onto
# standalone EventSemaphore instructions placed just before the original
# (same engine stream, so blocking semantics are identical).
# ---------------------------------------------------------------------------


def _split_multiwaits(bir_json: bytes) -> bytes:
    m = json.loads(bir_json)
    changed = False
    for func in m.get("functions", []):
        for blk in func.get("blocks", []) or []:
            insts = blk.get("instructions")
            if not insts:
                continue
            out = []
            for inst in insts:
                si = inst.get("sync_info") or {}
                waits = si.get("on_wait") or []
                if len(waits) > 1:
                    for i, w in enumerate(waits[:-1]):
                        out.append(
                            {
                                "debug": inst.get("debug", 0),
                                "engine": inst["engine"],
                                "ins": [],
                                "name": f"{inst['name']}__w{i}",
                                "opcode": "EventSemaphore",
                                "outs": [],
                                "sync_info": {"on_update": [], "on_wait": [w]},
                            }
                        )
                    si["on_wait"] = [waits[-1]]
                    changed = True
                out.append(inst)
            blk["instructions"] = out
    return json.dumps(m).encode() if changed else bir_json


_patched = False


def _install_patch():
    global _patched
    if _patched:
        return
    import concourse.bass_utils as bass_utils

    orig = bass_utils.compile_bir_kernel

    def patched(bir_json, tmpdir, neff_name="file.neff"):
        return orig(_split_multiwaits(bir_json), tmpdir, neff_name=neff_name)

    bass_utils.compile_bir_kernel = patched
    try:
        import concourse.bass2jax as bass2jax

        bass2jax.compile_bir_kernel = patched
    except ImportError:
        pass
    _patched = True


# ---------------------------------------------------------------------------
# Problem constants (hardcoded per the task contract)
# ---------------------------------------------------------------------------

P = 128
D = 256  # input feature dim
H = 256  # hidden dim
MQ = 1024  # query rows
NK = 32768  # total key rows
NCORES = 8
NLOC = NK // NCORES  # 4096 key rows per core
CH = 512  # free-dim chunk size (matmul moving max / one PSUM bank)
NCH = NLOC // CH  # 8 chunks per core
F16 = mybir.dt.float16
F32 = mybir.dt.float32
# group plan: (start_col, width) sections; smaller tail sections shorten the
# final output-DMA drain
PLAN = [(0, 1024), (1024, 1024), (2048, 1024), (3072, 512), (3584, 512)]
PLAN_BURST = [(0, 2048), (2048, 1024), (3072, 512), (3584, 512)]


def _build_nc(reps=None, plan=None):
    """Build the single-core SPMD Bass program.

    Layout: everything transposed (features on partitions, sequence on the
    free dim). PSUM allocated as [128, 1024] pairs (2 banks); matmuls write
    512-wide halves, ACT/DVE drain whole pairs. Per chunk-group (CG=2
    chunks of 512): prefix-scan chunk -> MLP1 -> MLP2 -> scores -> DMA out,
    so output DMA starts early and overlaps the remaining compute.

    reps: when set (>1), wraps the body in a For_i repeat loop — used only
    by the timing harness to measure per-iteration HW time.
    """
    _install_patch()
    from contextlib import ExitStack

    Relu = mybir.ActivationFunctionType.Relu
    Ident = mybir.ActivationFunctionType.Identity
    Max = mybir.AluOpType.max

    nc = bass.Bass()
    keyT = nc.declare_dram_parameter("keyT", [D, NLOC], F16, isOutput=False)
    queryT = nc.declare_dram_parameter("queryT", [D, MQ], F16, isOutput=False)
    k1_wT = nc.declare_dram_parameter("k1_wT", [3 * D, H], F16, isOutput=False)
    k2_wT = nc.declare_dram_parameter("k2_wT", [H, H], F16, isOutput=False)
    q1_wT = nc.declare_dram_parameter("q1_wT", [D, H], F16, isOutput=False)
    q2_wT = nc.declare_dram_parameter("q2_wT", [H, H], F16, isOutput=False)
    # per-core vectors: [:,0]=k1_b [:,1]=k2_b [:,2]=q1_b [:,3]=q2_b/16
    #                   [:,4]=before_seed [:,5]=after_seed
    vecs = nc.declare_dram_parameter("vecs", [D, 8], F32, isOutput=False)
    # per-core boundary columns: [:,0]=before col0, [:,1]=after last col
    cols = nc.declare_dram_parameter("cols", [D, 2], F16, isOutput=False)
    out = nc.declare_dram_parameter("out", [MQ, NLOC], F32, isOutput=True)

    PAIR = 2 * CH  # 1024
    NCG = NLOC // PAIR  # 4 prefix-scan chunks
    if plan is None:
        plan = PLAN

    with tile.TileContext(nc) as tc, ExitStack() as ctx:
        cpool = ctx.enter_context(tc.tile_pool(name="const", bufs=1))
        bpool = ctx.enter_context(tc.tile_pool(name="big", bufs=1))
        opool = ctx.enter_context(tc.tile_pool(name="outs", bufs=6))
        pspool = ctx.enter_context(
            tc.tile_pool(name="ps", bufs=4, space=bass.MemorySpace.PSUM)
        )
        if reps and reps > 1:
            E = mybir.EngineType
            ctx.enter_context(
                tc.For_i(0, reps, 1, hint_engines=(E.PE, E.Activation, E.DVE, E.SP))
            )

        # ---- key tiles first on the SP HWDGE queue (scans gate everything);
        #      constants + query path go on the gpsimd SWDGE queue in parallel
        kT = [bpool.tile([P, NLOC], F16, tag=f"kT{t}", name=f"kT{t}") for t in range(2)]
        nc.sync.dma_start(kT[0][:], keyT[0:P, :])
        nc.scalar.dma_start(kT[1][:], keyT[P : 2 * P, :])
        vec_sb = [cpool.tile([P, 8], F32, tag=f"vec{t}", name=f"vec{t}") for t in range(2)]
        cols_sb = [cpool.tile([P, 2], F16, tag=f"cols{t}", name=f"cols{t}") for t in range(2)]
        wq1 = [cpool.tile([P, H], F16, tag=f"wq1_{i}", name=f"wq1_{i}") for i in range(2)]
        wq2 = [cpool.tile([P, H], F16, tag=f"wq2_{i}", name=f"wq2_{i}") for i in range(2)]
        qT = [bpool.tile([P, MQ], F16, tag=f"qT{t}", name=f"qT{t}") for t in range(2)]
        wk1 = [cpool.tile([P, H], F16, tag=f"wk1_{i}", name=f"wk1_{i}") for i in range(6)]
        wk2 = [cpool.tile([P, H], F16, tag=f"wk2_{i}", name=f"wk2_{i}") for i in range(2)]
        for t in range(2):
            nc.scalar.dma_start(vec_sb[t][:], vecs[t * P : (t + 1) * P, :])
            nc.scalar.dma_start(cols_sb[t][:], cols[t * P : (t + 1) * P, :])
            nc.scalar.dma_start(wq1[t][:], q1_wT[t * P : (t + 1) * P, :])
            nc.scalar.dma_start(wq2[t][:], q2_wT[t * P : (t + 1) * P, :])
            nc.sync.dma_start(qT[t][:], queryT[t * P : (t + 1) * P, :])
        for i in range(6):
            nc.scalar.dma_start(wk1[i][:], k1_wT[i * P : (i + 1) * P, :])
        for i in range(2):
            nc.scalar.dma_start(wk2[i][:], k2_wT[i * P : (i + 1) * P, :])

        # ---- query MLP: xqT[h] = [128, 1024] f16 (one PSUM pair per h/layer)
        qh1 = [bpool.tile([P, MQ], F16, tag=f"qh1_{t}", name=f"qh1_{t}") for t in range(2)]
        xqT = [bpool.tile([P, MQ], F16, tag=f"xqT{t}", name=f"xqT{t}") for t in range(2)]
        for h in range(2):
            hs = slice(h * P, (h + 1) * P)
            ps = pspool.tile([P, PAIR], F32, tag="ps", name="ps")
            for kc in range(2):
                for c in range(2):
                    nc.tensor.matmul(
                        ps[:, c * CH : (c + 1) * CH], wq1[kc][:, hs],
                        qT[kc][:, c * CH : (c + 1) * CH],
                        start=(kc == 0), stop=(kc == 1),
                    )
            nc.scalar.activation(qh1[h][:], ps[:], Relu, bias=vec_sb[h][:, 2:3])
        for h in range(2):
            hs = slice(h * P, (h + 1) * P)
            ps = pspool.tile([P, PAIR], F32, tag="ps", name="ps")
            for kc in range(2):
                for c in range(2):
                    nc.tensor.matmul(
                        ps[:, c * CH : (c + 1) * CH], wq2[kc][:, hs],
                        qh1[kc][:, c * CH : (c + 1) * CH],
                        start=(kc == 0), stop=(kc == 1),
                    )
            nc.scalar.activation(xqT[h][:], ps[:], Ident, bias=vec_sb[h][:, 3:4])

        # ---- scans
        # befT[:, j] holds max(seed, key[0..j-1]) for j>=1; col 0 = host col.
        # aftT[:, j+1] holds max(seed, key[j..n-1]); col NLOC = host col, so
        # after = aftT[:, 1:NLOC+1].
        befT = [bpool.tile([P, NLOC + 1], F16, tag=f"befT{t}", name=f"befT{t}") for t in range(2)]
        aftT = [bpool.tile([P, NLOC + 1], F16, tag=f"aftT{t}", name=f"aftT{t}") for t in range(2)]
        for t in range(2):
            nc.vector.tensor_copy(befT[t][:, 0:1], cols_sb[t][:, 0:1])
            nc.vector.tensor_copy(aftT[t][:, NLOC : NLOC + 1], cols_sb[t][:, 1:2])
        # suffix scans first (whole row, reversed): every chunk-group needs them
        for t in range(2):
            rev_in = kT[t][:][:, ::-1]
            nc.vector.tensor_tensor_scan(
                aftT[t][:, 0:NLOC][:, ::-1], rev_in, rev_in,
                vec_sb[t][:, 5:6], op0=Max, op1=Max,
            )

        # xT feature rows for MLP1 (K = 768): 0-255 key | 256-511 bef | 512-767 aft
        def rhs1(kc, lo, hi):
            if kc < 2:
                return kT[kc][:, lo:hi]
            if kc < 4:
                return befT[kc - 2][:, lo:hi]
            return aftT[kc - 4][:, lo + 1 : hi + 1]

        # ---- per group: cover scans -> MLP1 -> MLP2 -> scores -> out
        h1 = [bpool.tile([P, NLOC], F16, tag=f"h1_{t}", name=f"h1_{t}") for t in range(2)]
        xkT = [bpool.tile([P, NLOC], F16, tag=f"xkT{t}", name=f"xkT{t}") for t in range(2)]

        scan_done = [False] * NCG

        def cover_scans(lo, w):
            for cg in range(lo // PAIR, (lo + w + PAIR - 1) // PAIR):
                if scan_done[cg]:
                    continue
                scan_done[cg] = True
                g0 = cg * PAIR
                for t in range(2):
                    init = vec_sb[t][:, 4:5] if cg == 0 else befT[t][:, g0 : g0 + 1]
                    nc.vector.tensor_tensor_scan(
                        befT[t][:, g0 + 1 : g0 + PAIR + 1],
                        kT[t][:, g0 : g0 + PAIR], kT[t][:, g0 : g0 + PAIR],
                        init, op0=Max, op1=Max,
                    )

        cover_scans(*plan[0])
        for gi, (lo, w) in enumerate(plan):
            ntiles = (w + PAIR - 1) // PAIR
            widths = [min(PAIR, w - i * PAIR) for i in range(ntiles)]
            offs = [lo + i * PAIR for i in range(ntiles)]

            def group_matmuls(pss, wtiles, nkc, rhs_of):
                for kc in range(nkc):
                    for i in range(ntiles):
                        for c in range(widths[i] // CH):
                            a = offs[i] + c * CH
                            nc.tensor.matmul(
                                pss[i][:, c * CH : (c + 1) * CH], wtiles(kc),
                                rhs_of(kc, a, a + CH),
                                start=(kc == 0), stop=(kc == nkc - 1),
                            )

            # MLP1: h1 = relu(k1_wT.T @ [key|bef|aft] + k1_b)
            for h in range(2):
                hs = slice(h * P, (h + 1) * P)
                pss = [pspool.tile([P, widths[i]], F32, tag="ps", name="ps")
                       for i in range(ntiles)]
                group_matmuls(pss, lambda kc: wk1[kc][:, hs], 6, rhs1)
                for i in range(ntiles):
                    nc.scalar.activation(
                        h1[h][:, offs[i] : offs[i] + widths[i]], pss[i][:],
                        Relu, bias=vec_sb[h][:, 0:1],
                    )
            # MLP2: xkT = k2_wT.T @ h1 + k2_b
            for h in range(2):
                hs = slice(h * P, (h + 1) * P)
                pss = [pspool.tile([P, widths[i]], F32, tag="ps", name="ps")
                       for i in range(ntiles)]
                group_matmuls(
                    pss, lambda kc: wk2[kc][:, hs], 2,
                    lambda kc, a, b: h1[kc][:, a:b],
                )
                for i in range(ntiles):
                    nc.scalar.activation(
                        xkT[h][:, offs[i] : offs[i] + widths[i]], pss[i][:],
                        Ident, bias=vec_sb[h][:, 1:2],
                    )
            if gi + 1 < len(plan):
                cover_scans(*plan[gi + 1])  # DVE runs it under this group's scores
            # scores: out[m, lo:lo+w] = xqT.T @ xkT (pre-scaled by 1/16)
            for m in range(MQ // P):
                ms = slice(m * P, (m + 1) * P)
                pss = [pspool.tile([P, widths[i]], F32, tag="ps", name="ps")
                       for i in range(ntiles)]
                group_matmuls(
                    pss, lambda kc: xqT[kc][:, ms], 2,
                    lambda kc, a, b: xkT[kc][:, a:b],
                )
                for i in range(ntiles):
                    ot = opool.tile([P, widths[i]], F32, tag="ot", name="ot")
                    if m % 2 == 0:
                        nc.scalar.copy(ot[:], pss[i][:])
                    else:
                        nc.vector.tensor_copy(ot[:], pss[i][:])
                    eng = nc.sync if m % 2 == 0 else nc.scalar
                    eng.dma_start(out[ms, offs[i] : offs[i] + widths[i]], ot[:])
    return nc


_nc_cache = None


def _get_nc():
    global _nc_cache
    if _nc_cache is None:
        _nc_cache = _build_nc()
    return _nc_cache


def _prep_in_maps(query, key, q1_w, q1_b, q2_w, q2_b, k1_w, k1_b, k2_w, k2_b):
    """Host-side sharding prep: transpose/cast to bf16, per-shard cummax seeds."""
    bf = np.float16
    key_bf = np.asarray(key, np.float32).astype(bf)  # [NK, D]
    keyT_bf = np.ascontiguousarray(key_bf.T)  # [D, NK]
    queryT = np.ascontiguousarray(np.asarray(query, np.float32).T).astype(bf)

    k1_wT = np.ascontiguousarray(np.asarray(k1_w, np.float32).T).astype(bf)
    k2_wT = np.ascontiguousarray(np.asarray(k2_w, np.float32).T).astype(bf)
    q1_wT = np.ascontiguousarray(np.asarray(q1_w, np.float32).T).astype(bf)
    q2_wT = np.ascontiguousarray(np.asarray(q2_w, np.float32).T / 16.0).astype(bf)

    # per-shard maxima of the bf16-rounded keys (exact in bf16)
    km = key_bf.astype(np.float32).reshape(NCORES, NLOC, D).max(axis=1)  # [8, D]
    NEG = -60000.0  # fp16-exact, far below any data value
    before_seed = np.full((NCORES, D), NEG, np.float32)
    after_seed = np.full((NCORES, D), NEG, np.float32)
    for s in range(1, NCORES):
        before_seed[s] = km[:s].max(axis=0)
    for s in range(NCORES - 1):
        after_seed[s] = km[s + 1 :].max(axis=0)
    before_col0 = before_seed.copy()
    before_col0[0] = 0.0
    after_col = after_seed.copy()
    after_col[NCORES - 1] = 0.0

    in_maps = []
    for s in range(NCORES):
        vecs = np.zeros((D, 8), np.float32)
        vecs[:, 0] = np.asarray(k1_b, np.float32)
        vecs[:, 1] = np.asarray(k2_b, np.float32)
        vecs[:, 2] = np.asarray(q1_b, np.float32)
        vecs[:, 3] = np.asarray(q2_b, np.float32) / 16.0
        vecs[:, 4] = before_seed[s]
        vecs[:, 5] = after_seed[s]
        cols = np.zeros((D, 2), np.float32)
        cols[:, 0] = before_col0[s]
        cols[:, 1] = after_col[s]
        in_maps.append(
            {
                "keyT": np.ascontiguousarray(keyT_bf[:, s * NLOC : (s + 1) * NLOC]),
                "queryT": queryT,
                "k1_wT": k1_wT,
                "k2_wT": k2_wT,
                "q1_wT": q1_wT,
                "q2_wT": q2_wT,
                "vecs": vecs,
                "cols": cols.astype(bf),
            }
        )
    return in_maps


def kernel(**inputs):
    from concourse.bass_utils import run_bass_kernel_spmd

    nc = _get_nc()
    in_maps = _prep_in_maps(**inputs)
    res = run_bass_kernel_spmd(nc, in_maps, list(range(NCORES)))
    return np.concatenate([r["out"] for r in res.results], axis=1)

